# revision 66
# baseline (speedup 1.0000x reference)
"""Trainium2 Bass kernel for nn_AttentionHead (causal single-head attention
with input projections), data-parallel over the batch dim on 8 NeuronCores.

Per-core computation (batch b):
  qh = q[b] @ Wq ; kh = k[b] @ Wk ; vh = v[b] @ Wv        [2048, 64]
  scores = (qh @ kh^T) * 8, causal-masked, softmax over s
  out[b] = softmax(scores) @ vh                            [2048, 64]

Implementation notes:
  - Host pre-transposes q/k/v to [E, L] layout (e on partitions) so the
    projection contraction dim lands on SBUF partitions, and casts to fp16:
    the PE rounds matmul operands to ~11 mantissa bits anyway, so fp16 inputs
    lose nothing while halving HBM traffic (accumulation stays fp32 in PSUM).
  - Load/compute schedule: k and q stream first and all 16 score/softmax
    tiles (phase1) overlap that stream; v streams last and the cheap AV
    matmuls (phase2) ride on its tail, so the DMA pipe never idles.
  - Wq is scaled by -8 on the host: the QK matmul then directly produces
    n = -8*scores, so the softmax bias (-8*rowmax) is exactly reduce_min(n)
    and exp uses scale=-1, with zero extra bias-prep ops.
  - Precision recovery on top of fp16 inputs: Wq and Wk ship as fp16 hi+lo
    pairs (extra lhsT columns, ~22-bit weights, no extra PE cycles); the
    projected qh^T is stored as an fp32r hi/lo pair stacked on the QK
    contraction partitions (lhsT = [qh_hi; qh_lo], rhs = [kh; kh]) so the
    q-side store rounding cancels; kh^T is stored fp32r (12-bit, RNE).
    Measured end-to-end rel err ~2.6e-3 vs the fp32 reference.
  - Softmax: exact row min of n (DVE, read from PSUM), exp fused on ScalarE
    reading PSUM directly, output fp16 (values in (0, 1]).
  - P^T and vh^T via PE transpose (fp16, exact), grouped 4 per PSUM bank
    with one copy per group (DMA-transpose was exact but intermittently hit
    the XBAR transpose/copy hang under concurrent DMA traffic, so PE only).
  - AV matmul in fp16 with a ones-column appended to vh so the softmax
    denominator Z falls out of the same accumulation (column 64 of the
    [l, d+1] output); final out = pav[:, :64] / Z, no output transpose.
"""
import sys

if "/opt/trn_rl_repo" not in sys.path:
    sys.path.insert(0, "/opt/trn_rl_repo")

import numpy as np

N_CORES = 8
NB, L, S, E, D = 8, 2048, 2048, 1024, 64
P = 128
ECH = E // P          # 8 e-chunks
LCH = 4               # l/s chunks of 512 for projections
NLT = L // P          # 16 l-tiles
NST = S // P          # 16 s-tiles
CHUNK = 512

_PROGRAM = None


def _build_program():
    import concourse.bacc as bacc
    import concourse.mybir as mybir
    import concourse.tile as tile
    from concourse.bass import ds

    F32 = mybir.dt.float32
    F16 = mybir.dt.float16
    F32R = mybir.dt.float32r
    Exp = mybir.ActivationFunctionType.Exp
    AX = mybir.AxisListType.X

    nc = bacc.Bacc(None, target_bir_lowering=False)

    kT = nc.declare_dram_parameter("kT", [P, ECH, S], F16, isOutput=False)
    qT = nc.declare_dram_parameter("qT", [P, ECH, L], F16, isOutput=False)
    vT = nc.declare_dram_parameter("vT", [P, ECH, S], F16, isOutput=False)
    Wk_s = nc.declare_dram_parameter("Wk_s", [P, ECH, 2 * D], F16, isOutput=False)
    Wq_s = nc.declare_dram_parameter("Wq_s", [P, ECH, 2 * D], F16, isOutput=False)
    Wv = nc.declare_dram_parameter("Wv", [P, ECH, D], F16, isOutput=False)
    id16_d = nc.declare_dram_parameter("id16", [P, P], F16, isOutput=False)
    dmask_d = nc.declare_dram_parameter("dmask", [P, P], F32, isOutput=False)
    out_d = nc.declare_dram_parameter("out", [L, D], F32, isOutput=True)

    with tile.TileContext(nc) as tc:
        with (
            tc.tile_pool(name="consts", bufs=1) as consts,
            tc.tile_pool(name="persist", bufs=1) as persist,
            tc.tile_pool(name="xstream", bufs=8) as xstream,
            tc.tile_pool(name="work", bufs=3) as work,
            tc.tile_pool(name="epool", bufs=4) as epool,
            tc.tile_pool(name="etpool", bufs=40) as etpool,
            tc.tile_pool(name="psBig", bufs=4, space="PSUM") as psBig,
            tc.tile_pool(name="psC", bufs=2, space="PSUM") as psC,
            tc.tile_pool(name="psD", bufs=2, space="PSUM") as psD,
        ):
            # ---- constants ----
            wk_t = consts.tile([P, ECH, 2 * D], F16, tag="wk")
            wq_t = consts.tile([P, ECH, 2 * D], F16, tag="wq")
            wv_t = consts.tile([P, ECH, D], F16, tag="wv")
            id16_t = consts.tile([P, P], F16, tag="id16")
            dmask_t = consts.tile([P, P], F32, tag="dmask")
            nc.sync.dma_start(out=wk_t, in_=Wk_s[:])
            nc.sync.dma_start(out=wq_t, in_=Wq_s[:])

            # ---- persistent projected tensors (fp32r: 12-bit stores) ----
            # qsplit: rows 0-63 = r12(-8*qh^T hi), rows 64-127 = residual lo
            qsplit = persist.tile([P, L], F32R, tag="qsp", name="qsp")
            # kdup[c]: rows 0-63 = r12(kh^T), rows 64-127 = copy
            kdup = [persist.tile([P, CHUNK], F32R, tag=f"kd{c}", name=f"kd{c}")
                    for c in range(LCH)]
            # vones[:, j, :]: [128, 65]; cols 0-63 = vh rows, col 64 = 1.0
            vones = persist.tile([P, NST, D + 1], F16, tag="vo", name="vo")
            nc.gpsimd.memset(vones[:, :, D : D + 1], 1.0)

            def proj_k(lc):
                """k projection, W hi/lo split (M=128); kh stored fp32r.

                Loaded and projected in two 256-col halves so the PE and the
                epilogue pipeline against the chunk's DMA arrival.
                """
                H = CHUNK // 2
                kt = xstream.tile([P, ECH, CHUNK], F16, tag="xs", name="kt")
                ps = psBig.tile([P, CHUNK], F32, tag="big")
                kd = kdup[lc]
                for h in range(2):
                    hs = ds(h * H, H)
                    nc.sync.dma_start(
                        out=kt[:, :, hs],
                        in_=kT[:, :, ds(lc * CHUNK + h * H, H)],
                    )
                    for c in range(ECH):
                        nc.tensor.matmul(
                            ps[:, hs], wk_t[:, c, :], kt[:, c, hs],
                            start=(c == 0), stop=(c == ECH - 1),
                        )
                    lo_sb = work.tile([D, H], F32, tag="losb")
                    nc.scalar.copy(out=lo_sb, in_=ps[D:, hs])
                    nc.vector.tensor_add(
                        out=kd[:D, hs], in0=ps[:D, hs], in1=lo_sb
                    )
                    nc.gpsimd.tensor_copy(out=kd[D:, hs], in_=kd[:D, hs])

            def proj_q(lc):
                """q projection with -8 scale, W hi/lo split (M=128).

                A = hi-W product (psum rows 0-63), B = lo-W product (64-127);
                qh' = A + B. Store hi = r12(A), lo = r12(B + (A - hi)).
                """
                H = CHUNK // 2
                qt = xstream.tile([P, ECH, CHUNK], F16, tag="xs", name="qt")
                ps = psBig.tile([P, CHUNK], F32, tag="big")
                for h in range(2):
                    hs = ds(h * H, H)
                    nc.sync.dma_start(
                        out=qt[:, :, hs],
                        in_=qT[:, :, ds(lc * CHUNK + h * H, H)],
                    )
                    for c in range(ECH):
                        nc.tensor.matmul(
                            ps[:, hs], wq_t[:, c, :], qt[:, c, hs],
                            start=(c == 0), stop=(c == ECH - 1),
                        )
                    qsl = qsplit[:, ds(lc * CHUNK + h * H, H)]
                    nc.vector.tensor_copy(out=qsl[:D, :], in_=ps[:D, hs])
                    s2 = work.tile([D, H], F32, tag="qres")
                    nc.vector.tensor_tensor(
                        out=s2, in0=ps[:D, hs], in1=qsl[:D, :].bitcast(F32),
                        op=mybir.AluOpType.subtract,
                    )
                    nc.vector.tensor_tensor(
                        out=qsl[D:, :], in0=ps[D:, hs], in1=s2,
                        op=mybir.AluOpType.add,
                    )

            def proj_v(lc):
                """v projection (M=64), loaded/projected in 256-col halves."""
                H = CHUNK // 2
                vt = xstream.tile([P, ECH, CHUNK], F16, tag="xs", name="vt")
                ps = psBig.tile([P, CHUNK], F32, tag="big")
                for h in range(2):
                    hs = ds(h * H, H)
                    nc.sync.dma_start(
                        out=vt[:, :, hs],
                        in_=vT[:, :, ds(lc * CHUNK + h * H, H)],
                    )
                    for c in range(ECH):
                        nc.tensor.matmul(
                            ps[:D, hs], wv_t[:, c, :], vt[:, c, hs],
                            start=(c == 0), stop=(c == ECH - 1),
                        )
                    vh16 = work.tile([D, H], F16, tag="vtmp")
                    nc.scalar.copy(out=vh16, in_=ps[:D, hs])
                    pt4 = psC.tile([P, CHUNK], F16, tag="pt")
                    for j in range(2):
                        nc.tensor.transpose(
                            pt4[:, ds(j * P, P)][:, :D],
                            vh16[:, ds(j * P, P)], id16_t[:D, :D]
                        )
                    st0 = lc * 4 + h * 2
                    nc.scalar.copy(
                        out=vones[:, st0 : st0 + 2, :D],
                        in_=pt4[:, : 2 * P].rearrange(
                            "p (j d) -> p j d", j=2)[:, :, :D],
                    )

            def attn_phase1(i):
                """QK (n = -8*scores) -> mask -> row min -> exp -> E^T."""
                ncols = (i + 1) * P
                nch = (ncols + CHUNK - 1) // CHUNK
                dc, doff = i // 4, (i % 4) * P
                mins = work.tile([P, 4], F32, tag="mins")
                pscs = []
                for c2 in range(nch):
                    n = min(CHUNK, ncols - c2 * CHUNK)
                    psc = psBig.tile([P, CHUNK], F32, tag="big")
                    nc.tensor.matmul(
                        psc[:, :n], qsplit[:, ds(i * P, P)], kdup[c2][:, :n],
                        start=True, stop=True,
                    )
                    if c2 == dc:
                        nc.vector.tensor_add(
                            out=psc[:, ds(doff, P)], in0=psc[:, ds(doff, P)],
                            in1=dmask_t,
                        )
                    nc.vector.tensor_reduce(
                        out=mins[:, ds(c2, 1)], in_=psc[:, :n], axis=AX,
                        op=mybir.AluOpType.min,
                    )
                    pscs.append((psc, n))
                bm = work.tile([P, 1], F32, tag="bm")
                nc.vector.tensor_reduce(
                    out=bm, in_=mins[:, :nch], axis=AX, op=mybir.AluOpType.min
                )
                ets = []
                for c2, (psc, n) in enumerate(pscs):
                    ec = epool.tile([P, CHUNK], F16, tag="E")
                    nc.scalar.activation(
                        out=ec[:, :n], in_=psc[:, :n],
                        func=Exp, bias=bm, scale=-1.0,
                    )
                    nblk = n // P
                    pt4 = psC.tile([P, CHUNK], F16, tag="pt")
                    for jj in range(nblk):
                        nc.tensor.transpose(
                            pt4[:, ds(jj * P, P)], ec[:, ds(jj * P, P)], id16_t
                        )
                    et4 = etpool.tile([P, CHUNK], F16, tag="et")
                    if (i + c2) % 3 != 0:
                        nc.scalar.copy(out=et4[:, :n], in_=pt4[:, :n])
                    else:
                        nc.vector.tensor_copy(out=et4[:, :n], in_=pt4[:, :n])
                    ets.append((et4, nblk))
                return ets

            def attn_phase2(i, ets):
                """AV+Z matmul from saved E^T tiles -> normalize -> DMA out."""
                pav = psD.tile([P, D + 1], F32, tag="pav")
                first = True
                for c2, (et4, nblk) in enumerate(ets):
                    for jj in range(nblk):
                        j = c2 * 4 + jj
                        nc.tensor.matmul(
                            pav, et4[:, ds(jj * P, P)], vones[:, j, :],
                            start=first,
                            stop=(c2 == len(ets) - 1 and jj == nblk - 1),
                        )
                        first = False
                zi = work.tile([P, 1], F32, tag="zi")
                nc.vector.reciprocal(zi, pav[:, D : D + 1])
                ob = work.tile([P, D], F32, tag="ob")
                nc.vector.tensor_scalar_mul(ob, pav[:, :D], zi)
                nc.sync.dma_start(out=out_d[ds(i * P, P), :], in_=ob)

            # k+q stream first; all score/softmax work (phase1) overlaps the
            # stream. v projections are emitted one super-iter behind and the
            # cheap AV passes (phase2) trail phase1 by PIPE tiles so the
            # in-order engine queues never stall on late v data.
            PIPE = 6
            saved = {}
            for lc in range(LCH):
                proj_k(lc)
                proj_q(lc)
                if lc == 0:
                    nc.sync.dma_start(out=id16_t, in_=id16_d[:])
                    nc.sync.dma_start(out=dmask_t, in_=dmask_d[:])
                    nc.sync.dma_start(out=wv_t, in_=Wv[:])
                else:
                    proj_v(lc - 1)
                for j in range(4):
                    i = lc * 4 + j
                    saved[i] = attn_phase1(i)
                    if i - PIPE >= 0:
                        attn_phase2(i - PIPE, saved.pop(i - PIPE))
            proj_v(LCH - 1)
            for i in range(NLT - PIPE, NLT):
                attn_phase2(i, saved.pop(i))

    nc.finalize()
    return nc


def _get_program():
    global _PROGRAM
    if _PROGRAM is None:
        _PROGRAM = _build_program()
    return _PROGRAM


def make_in_maps(q, k, v, Wq, Wk, Wv):
    """Host-side sharding + layout prep. Returns one input map per core."""
    def w_split(W):
        W = np.asarray(W, dtype=np.float32)
        hi = W.astype(np.float16)
        lo = (W - hi.astype(np.float32)).astype(np.float16)
        return np.ascontiguousarray(
            np.concatenate([hi, lo], axis=1).reshape(ECH, P, 2 * D)
            .transpose(1, 0, 2)
        )

    wk_s = w_split(Wk)
    wq_s = w_split(np.asarray(Wq, np.float32) * np.float32(-8.0))
    wv = np.ascontiguousarray(
        np.asarray(Wv, np.float32).astype(np.float16)
        .reshape(ECH, P, D).transpose(1, 0, 2)
    )
    id16 = np.eye(P, dtype=np.float16)
    # masked (s > l within the diagonal block) -> +1e30 in n = -8*scores
    dmask = np.where(
        np.arange(P)[None, :] > np.arange(P)[:, None], np.float32(1e30), np.float32(0)
    ).astype(np.float32)

    in_maps = []
    for b in range(N_CORES):
        def xt(x):
            return np.ascontiguousarray(
                np.asarray(x, dtype=np.float32).T
                .reshape(ECH, P, -1).transpose(1, 0, 2)
            ).astype(np.float16)

        in_maps.append({
            "qT": xt(q[b]), "kT": xt(k[b]), "vT": xt(v[b]),
            "Wk_s": wk_s, "Wq_s": wq_s, "Wv": wv,
            "id16": id16, "dmask": dmask,
        })
    return in_maps


def kernel(q, k, v, Wq, Wk, Wv, attn_mask=None):
    from concourse.bass_utils import run_bass_kernel_spmd

    nc = _get_program()
    in_maps = make_in_maps(q, k, v, Wq, Wk, Wv)
    res = run_bass_kernel_spmd(nc, in_maps, core_ids=list(range(N_CORES)))
    out = np.stack([res.results[b]["out"] for b in range(N_CORES)], axis=0)
    return out.astype(np.float32)


# revision 71
# speedup vs baseline: 1.0068x; 1.0068x over previous
"""Trainium2 Bass kernel for nn_AttentionHead (causal single-head attention
with input projections), data-parallel over the batch dim on 8 NeuronCores.

Per-core computation (batch b):
  qh = q[b] @ Wq ; kh = k[b] @ Wk ; vh = v[b] @ Wv        [2048, 64]
  scores = (qh @ kh^T) * 8, causal-masked, softmax over s
  out[b] = softmax(scores) @ vh                            [2048, 64]

Implementation notes:
  - Host pre-transposes q/k/v to [E, L] layout (e on partitions) so the
    projection contraction dim lands on SBUF partitions, and casts to fp16:
    the PE rounds matmul operands to ~11 mantissa bits anyway, so fp16 inputs
    lose nothing while halving HBM traffic (accumulation stays fp32 in PSUM).
  - Load/compute schedule: k and q stream first and all 16 score/softmax
    tiles (phase1) overlap that stream; v streams last and the cheap AV
    matmuls (phase2) ride on its tail, so the DMA pipe never idles.
  - Wq is scaled by -8 on the host: the QK matmul then directly produces
    n = -8*scores, so the softmax bias (-8*rowmax) is exactly reduce_min(n)
    and exp uses scale=-1, with zero extra bias-prep ops.
  - Precision recovery on top of fp16 inputs: Wq and Wk ship as fp16 hi+lo
    pairs (extra lhsT columns, ~22-bit weights, no extra PE cycles); the
    projected qh^T is stored as an fp32r hi/lo pair stacked on the QK
    contraction partitions (lhsT = [qh_hi; qh_lo], rhs = [kh; kh]) so the
    q-side store rounding cancels; kh^T is stored fp32r (12-bit, RNE).
    Measured end-to-end rel err ~2.6e-3 vs the fp32 reference.
  - Softmax: exact row min of n (DVE, read from PSUM), exp fused on ScalarE
    reading PSUM directly, output fp16 (values in (0, 1]).
  - P^T and vh^T via PE transpose (fp16, exact), grouped 4 per PSUM bank
    with one copy per group (DMA-transpose was exact but intermittently hit
    the XBAR transpose/copy hang under concurrent DMA traffic, so PE only).
  - AV matmul in fp16 with a ones-column appended to vh so the softmax
    denominator Z falls out of the same accumulation (column 64 of the
    [l, d+1] output); final out = pav[:, :64] / Z, no output transpose.
"""
import sys

if "/opt/trn_rl_repo" not in sys.path:
    sys.path.insert(0, "/opt/trn_rl_repo")

import numpy as np

N_CORES = 8
NB, L, S, E, D = 8, 2048, 2048, 1024, 64
P = 128
ECH = E // P          # 8 e-chunks
LCH = 4               # l/s chunks of 512 for projections
NLT = L // P          # 16 l-tiles
NST = S // P          # 16 s-tiles
CHUNK = 512

_PROGRAM = None


def _build_program():
    import concourse.bacc as bacc
    import concourse.mybir as mybir
    import concourse.tile as tile
    from concourse.bass import ds

    F32 = mybir.dt.float32
    F16 = mybir.dt.float16
    F32R = mybir.dt.float32r
    Exp = mybir.ActivationFunctionType.Exp
    AX = mybir.AxisListType.X

    nc = bacc.Bacc(None, target_bir_lowering=False)

    kT = nc.declare_dram_parameter("kT", [P, ECH, S], F16, isOutput=False)
    qT = nc.declare_dram_parameter("qT", [P, ECH, L], F16, isOutput=False)
    vT = nc.declare_dram_parameter("vT", [P, ECH, S], F16, isOutput=False)
    Wk_s = nc.declare_dram_parameter("Wk_s", [P, ECH, 2 * D], F16, isOutput=False)
    Wq_s = nc.declare_dram_parameter("Wq_s", [P, ECH, 2 * D], F16, isOutput=False)
    Wv = nc.declare_dram_parameter("Wv", [P, ECH, D], F16, isOutput=False)
    id16_d = nc.declare_dram_parameter("id16", [P, P], F16, isOutput=False)
    dmask_d = nc.declare_dram_parameter("dmask", [P, P], F32, isOutput=False)
    out_d = nc.declare_dram_parameter("out", [L, D], F32, isOutput=True)

    with tile.TileContext(nc) as tc:
        with (
            tc.tile_pool(name="consts", bufs=1) as consts,
            tc.tile_pool(name="persist", bufs=1) as persist,
            tc.tile_pool(name="xstream", bufs=8) as xstream,
            tc.tile_pool(name="work", bufs=3) as work,
            tc.tile_pool(name="epool", bufs=4) as epool,
            tc.tile_pool(name="etpool", bufs=40) as etpool,
            tc.tile_pool(name="obpool", bufs=16) as obpool,
            tc.tile_pool(name="psBig", bufs=4, space="PSUM") as psBig,
            tc.tile_pool(name="psC", bufs=2, space="PSUM") as psC,
            tc.tile_pool(name="psD", bufs=2, space="PSUM") as psD,
        ):
            # ---- constants ----
            wk_t = consts.tile([P, ECH, 2 * D], F16, tag="wk")
            wq_t = consts.tile([P, ECH, 2 * D], F16, tag="wq")
            wv_t = consts.tile([P, ECH, D], F16, tag="wv")
            id16_t = consts.tile([P, P], F16, tag="id16")
            dmask_t = consts.tile([P, P], F32, tag="dmask")
            nc.sync.dma_start(out=wk_t, in_=Wk_s[:])
            nc.sync.dma_start(out=wq_t, in_=Wq_s[:])

            # ---- persistent projected tensors (fp32r: 12-bit stores) ----
            # qsplit: rows 0-63 = r12(-8*qh^T hi), rows 64-127 = residual lo
            qsplit = persist.tile([P, L], F32R, tag="qsp", name="qsp")
            # kdup[c]: rows 0-63 = r12(kh^T), rows 64-127 = copy
            kdup = [persist.tile([P, CHUNK], F32R, tag=f"kd{c}", name=f"kd{c}")
                    for c in range(LCH)]
            # vones[:, j, :]: [128, 65]; cols 0-63 = vh rows, col 64 = 1.0
            vones = persist.tile([P, NST, D + 1], F16, tag="vo", name="vo")
            nc.gpsimd.memset(vones[:, :, D : D + 1], 1.0)

            def proj_k(lc):
                """k projection, W hi/lo split (M=128); kh stored fp32r.

                Loaded and projected in two 256-col halves so the PE and the
                epilogue pipeline against the chunk's DMA arrival.
                """
                H = CHUNK // 2
                kt = xstream.tile([P, ECH, CHUNK], F16, tag="xs", name="kt")
                ps = psBig.tile([P, CHUNK], F32, tag="big")
                kd = kdup[lc]
                for h in range(2):
                    hs = ds(h * H, H)
                    nc.sync.dma_start(
                        out=kt[:, :, hs],
                        in_=kT[:, :, ds(lc * CHUNK + h * H, H)],
                    )
                    for c in range(ECH):
                        nc.tensor.matmul(
                            ps[:, hs], wk_t[:, c, :], kt[:, c, hs],
                            start=(c == 0), stop=(c == ECH - 1),
                        )
                    lo_sb = work.tile([D, H], F32, tag="losb")
                    nc.scalar.copy(out=lo_sb, in_=ps[D:, hs])
                    nc.vector.tensor_add(
                        out=kd[:D, hs], in0=ps[:D, hs], in1=lo_sb
                    )
                    nc.gpsimd.tensor_copy(out=kd[D:, hs], in_=kd[:D, hs])

            def proj_q(lc):
                """q projection with -8 scale, W hi/lo split (M=128).

                A = hi-W product (psum rows 0-63), B = lo-W product (64-127);
                qh' = A + B. Store hi = r12(A), lo = r12(B + (A - hi)).
                """
                H = CHUNK // 2
                qt = xstream.tile([P, ECH, CHUNK], F16, tag="xs", name="qt")
                ps = psBig.tile([P, CHUNK], F32, tag="big")
                for h in range(2):
                    hs = ds(h * H, H)
                    nc.sync.dma_start(
                        out=qt[:, :, hs],
                        in_=qT[:, :, ds(lc * CHUNK + h * H, H)],
                    )
                    for c in range(ECH):
                        nc.tensor.matmul(
                            ps[:, hs], wq_t[:, c, :], qt[:, c, hs],
                            start=(c == 0), stop=(c == ECH - 1),
                        )
                    qsl = qsplit[:, ds(lc * CHUNK + h * H, H)]
                    nc.vector.tensor_copy(out=qsl[:D, :], in_=ps[:D, hs])
                    s2 = work.tile([D, H], F32, tag="qres")
                    nc.vector.tensor_tensor(
                        out=s2, in0=ps[:D, hs], in1=qsl[:D, :].bitcast(F32),
                        op=mybir.AluOpType.subtract,
                    )
                    nc.vector.tensor_tensor(
                        out=qsl[D:, :], in0=ps[D:, hs], in1=s2,
                        op=mybir.AluOpType.add,
                    )

            def proj_v(lc):
                """v projection (M=64), loaded/projected in 256-col halves."""
                H = CHUNK // 2
                vt = xstream.tile([P, ECH, CHUNK], F16, tag="xs", name="vt")
                ps = psBig.tile([P, CHUNK], F32, tag="big")
                for h in range(2):
                    hs = ds(h * H, H)
                    nc.sync.dma_start(
                        out=vt[:, :, hs],
                        in_=vT[:, :, ds(lc * CHUNK + h * H, H)],
                    )
                    for c in range(ECH):
                        nc.tensor.matmul(
                            ps[:D, hs], wv_t[:, c, :], vt[:, c, hs],
                            start=(c == 0), stop=(c == ECH - 1),
                        )
                    vh16 = work.tile([D, H], F16, tag="vtmp")
                    nc.scalar.copy(out=vh16, in_=ps[:D, hs])
                    pt4 = psC.tile([P, CHUNK], F16, tag="pt")
                    for j in range(2):
                        nc.tensor.transpose(
                            pt4[:, ds(j * P, P)][:, :D],
                            vh16[:, ds(j * P, P)], id16_t[:D, :D]
                        )
                    st0 = lc * 4 + h * 2
                    nc.scalar.copy(
                        out=vones[:, st0 : st0 + 2, :D],
                        in_=pt4[:, : 2 * P].rearrange(
                            "p (j d) -> p j d", j=2)[:, :, :D],
                    )

            def attn_phase1(i):
                """QK (n = -8*scores) -> mask -> row min -> exp -> E^T."""
                ncols = (i + 1) * P
                nch = (ncols + CHUNK - 1) // CHUNK
                dc, doff = i // 4, (i % 4) * P
                mins = work.tile([P, 4], F32, tag="mins")
                pscs = []
                for c2 in range(nch):
                    n = min(CHUNK, ncols - c2 * CHUNK)
                    psc = psBig.tile([P, CHUNK], F32, tag="big")
                    nc.tensor.matmul(
                        psc[:, :n], qsplit[:, ds(i * P, P)], kdup[c2][:, :n],
                        start=True, stop=True,
                    )
                    if c2 == dc:
                        nc.vector.tensor_add(
                            out=psc[:, ds(doff, P)], in0=psc[:, ds(doff, P)],
                            in1=dmask_t,
                        )
                    nc.vector.tensor_reduce(
                        out=mins[:, ds(c2, 1)], in_=psc[:, :n], axis=AX,
                        op=mybir.AluOpType.min,
                    )
                    pscs.append((psc, n))
                bm = work.tile([P, 1], F32, tag="bm")
                nc.vector.tensor_reduce(
                    out=bm, in_=mins[:, :nch], axis=AX, op=mybir.AluOpType.min
                )
                ets = []
                for c2, (psc, n) in enumerate(pscs):
                    ec = epool.tile([P, CHUNK], F16, tag="E")
                    nc.scalar.activation(
                        out=ec[:, :n], in_=psc[:, :n],
                        func=Exp, bias=bm, scale=-1.0,
                    )
                    nblk = n // P
                    pt4 = psC.tile([P, CHUNK], F16, tag="pt")
                    for jj in range(nblk):
                        nc.tensor.transpose(
                            pt4[:, ds(jj * P, P)], ec[:, ds(jj * P, P)], id16_t
                        )
                    et4 = etpool.tile([P, CHUNK], F16, tag="et")
                    if (i + c2) % 3 != 0:
                        nc.scalar.copy(out=et4[:, :n], in_=pt4[:, :n])
                    else:
                        nc.vector.tensor_copy(out=et4[:, :n], in_=pt4[:, :n])
                    ets.append((et4, nblk))
                return ets

            def attn_phase2(i, ets):
                """AV+Z matmul from saved E^T tiles -> normalize to SBUF.
                The DMA out is deferred to the end of the SP queue so it
                never head-of-line blocks the input stream."""
                pav = psD.tile([P, D + 1], F32, tag="pav")
                first = True
                for c2, (et4, nblk) in enumerate(ets):
                    for jj in range(nblk):
                        j = c2 * 4 + jj
                        nc.tensor.matmul(
                            pav, et4[:, ds(jj * P, P)], vones[:, j, :],
                            start=first,
                            stop=(c2 == len(ets) - 1 and jj == nblk - 1),
                        )
                        first = False
                zi = work.tile([P, 1], F32, tag="zi")
                nc.vector.reciprocal(zi, pav[:, D : D + 1])
                ob = obpool.tile([P, D], F32, tag="ob")
                nc.vector.tensor_scalar_mul(ob, pav[:, :D], zi)
                obs.append((i, ob))

            # k+q stream first; all score/softmax work (phase1) overlaps the
            # stream. v projections are emitted one super-iter behind and the
            # cheap AV passes (phase2) trail phase1 by PIPE tiles so the
            # in-order engine queues never stall on late v data.
            PIPE = 4
            saved = {}
            obs = []
            for lc in range(LCH):
                proj_k(lc)
                proj_q(lc)
                if lc == 0:
                    nc.sync.dma_start(out=id16_t, in_=id16_d[:])
                    nc.sync.dma_start(out=dmask_t, in_=dmask_d[:])
                    nc.sync.dma_start(out=wv_t, in_=Wv[:])
                else:
                    proj_v(lc - 1)
                for j in range(4):
                    i = lc * 4 + j
                    saved[i] = attn_phase1(i)
                    if i - PIPE >= 0:
                        attn_phase2(i - PIPE, saved.pop(i - PIPE))
            proj_v(LCH - 1)
            for i in range(NLT - PIPE, NLT):
                attn_phase2(i, saved.pop(i))
            for i, ob in obs:
                nc.sync.dma_start(out=out_d[ds(i * P, P), :], in_=ob)

    nc.finalize()
    return nc


def _get_program():
    global _PROGRAM
    if _PROGRAM is None:
        _PROGRAM = _build_program()
    return _PROGRAM


def make_in_maps(q, k, v, Wq, Wk, Wv):
    """Host-side sharding + layout prep. Returns one input map per core."""
    def w_split(W):
        W = np.asarray(W, dtype=np.float32)
        hi = W.astype(np.float16)
        lo = (W - hi.astype(np.float32)).astype(np.float16)
        return np.ascontiguousarray(
            np.concatenate([hi, lo], axis=1).reshape(ECH, P, 2 * D)
            .transpose(1, 0, 2)
        )

    wk_s = w_split(Wk)
    wq_s = w_split(np.asarray(Wq, np.float32) * np.float32(-8.0))
    wv = np.ascontiguousarray(
        np.asarray(Wv, np.float32).astype(np.float16)
        .reshape(ECH, P, D).transpose(1, 0, 2)
    )
    id16 = np.eye(P, dtype=np.float16)
    # masked (s > l within the diagonal block) -> +1e30 in n = -8*scores
    dmask = np.where(
        np.arange(P)[None, :] > np.arange(P)[:, None], np.float32(1e30), np.float32(0)
    ).astype(np.float32)

    in_maps = []
    for b in range(N_CORES):
        def xt(x):
            return np.ascontiguousarray(
                np.asarray(x, dtype=np.float32).T
                .reshape(ECH, P, -1).transpose(1, 0, 2)
            ).astype(np.float16)

        in_maps.append({
            "qT": xt(q[b]), "kT": xt(k[b]), "vT": xt(v[b]),
            "Wk_s": wk_s, "Wq_s": wq_s, "Wv": wv,
            "id16": id16, "dmask": dmask,
        })
    return in_maps


def kernel(q, k, v, Wq, Wk, Wv, attn_mask=None):
    from concourse.bass_utils import run_bass_kernel_spmd

    nc = _get_program()
    in_maps = make_in_maps(q, k, v, Wq, Wk, Wv)
    res = run_bass_kernel_spmd(nc, in_maps, core_ids=list(range(N_CORES)))
    out = np.stack([res.results[b]["out"] for b in range(N_CORES)], axis=0)
    return out.astype(np.float32)
